# revision 1
# baseline (speedup 1.0000x reference)
"""Equiformer GNN message-passing kernel for 8 Trainium2 NeuronCores.

Strategy (self-contained; shapes derived from inputs):
  - Nodes partitioned into 8 contiguous chunks (balanced by incident-edge
    count); each core owns its chunk's nodes and all edges whose *dst* lies
    in the chunk (segment softmax / scatter stay core-local).
  - Edges sorted by dst, grouped into 128-node windows; scatter is done on
    the tensor engine via host-built 0/1 selector matrices into a PSUM
    window accumulator.
  - Per layer, each core computes LN + the Wv projection for its node
    chunk, writes a bf16 v-table chunk, and an AllGather replicates the
    full table; per-edge rows are fetched with dma_gather.
  - attn_a is folded into Wsh (am = |a|*m); logits come from a contiguous
    lrelu STT plus per-head sign-weighted accumulating STTs; 1/|a| is
    folded into Wo's rows.
  - All radial MLP tables are built in pairs (block-diagonal weights) and
    hoisted out of the attention edge phases so the scalar engine never
    thrashes activation tables between Silu and Exp.
  - Edge phase is software-pipelined: window w's messages/logits overlap
    window w-1's exp/softmax-weighting/scatter and window w+1's DMA.
"""

import os
import sys
import types
from contextlib import ExitStack

import numpy as np
import ml_dtypes

sys.path.insert(0, "/opt/trn_rl_repo")
sys.path.insert(0, "/root/.axon_site")

import concourse.bacc as bacc
import concourse.bass as bass
import concourse.mybir as mybir
import concourse.tile as tile
from concourse import library_config

BF16 = mybir.dt.bfloat16
F32 = mybir.dt.float32
I16 = mybir.dt.int16
AF = mybir.ActivationFunctionType
OP = mybir.AluOpType

NCORES = 8
H = 4
CUTOFF = 5.0
AVG_DEG = 16.0
AVG_NODES = 18.0
LN_EPS = 1e-5
SEG_EPS = 1e-9

_program_cache = {}


# ----------------------------------------------------------------------------
# host-side preprocessing
# ----------------------------------------------------------------------------

def _sph_l2_np(vec):
    r = np.linalg.norm(vec, axis=-1, keepdims=True)
    u = vec / (r + 1e-9)
    x, y, z = u[..., 0], u[..., 1], u[..., 2]
    s3, s15, s5 = np.sqrt(3.0), np.sqrt(15.0), np.sqrt(5.0)
    return np.stack([
        np.ones_like(x),
        s3 * x, s3 * y, s3 * z,
        s15 * x * y, s15 * y * z, 0.5 * s5 * (3.0 * z * z - 1.0),
        s15 * x * z, 0.5 * s15 * (x * x - y * y)], axis=-1).astype(np.float32)


def _rbf_np(d, nb):
    centers = np.linspace(0.0, CUTOFF, nb).astype(np.float32)
    w = CUTOFF / nb
    return np.exp(-0.5 * ((d[:, None] - centers[None, :]) / w) ** 2).astype(np.float32)


def _wrap_idx(idx):
    """int16 index array -> [128, n/16] wrapped layout for dma_gather."""
    n = idx.shape[0]
    assert n % 16 == 0
    w = np.zeros((16, n // 16), np.int16)
    for p in range(16):
        w[p, :] = idx[p::16]
    return np.tile(w, (8, 1))


def _prepare(inputs):
    z = np.asarray(inputs["z"]).astype(np.int64)
    pos = np.asarray(inputs["pos"]).astype(np.float32)
    batch = np.asarray(inputs["batch"]).astype(np.int64)
    esrc = np.asarray(inputs["edge_src"]).astype(np.int64)
    edst = np.asarray(inputs["edge_dst"]).astype(np.int64)
    atom_emb = np.asarray(inputs["atom_emb"]).astype(np.float32)
    W_deg_sh = np.asarray(inputs["W_deg_sh"]).astype(np.float32)
    deg_w1 = np.asarray(inputs["deg_w1"]).astype(np.float32)
    deg_w2 = np.asarray(inputs["deg_w2"]).astype(np.float32)
    deg_w3 = np.asarray(inputs["deg_w3"]).astype(np.float32)
    Wv = np.asarray(inputs["Wv"]).astype(np.float32)
    Wsh = np.asarray(inputs["Wsh"]).astype(np.float32)
    rad_w1 = np.asarray(inputs["rad_w1"]).astype(np.float32)
    rad_w2 = np.asarray(inputs["rad_w2"]).astype(np.float32)
    rad_w3 = np.asarray(inputs["rad_w3"]).astype(np.float32)
    attn_a = np.asarray(inputs["attn_a"]).astype(np.float32)
    Wo = np.asarray(inputs["Wo"]).astype(np.float32)
    ffn_w1 = np.asarray(inputs["ffn_w1"]).astype(np.float32)
    ffn_w2 = np.asarray(inputs["ffn_w2"]).astype(np.float32)
    head_w1 = np.asarray(inputs["head_w1"]).astype(np.float32)
    head_w2 = np.asarray(inputs["head_w2"]).astype(np.float32)

    N = z.shape[0]
    E = esrc.shape[0]
    D = atom_emb.shape[1]
    SH = Wsh.shape[1]
    NB = deg_w1.shape[0]
    FCH = deg_w1.shape[1]
    L = Wv.shape[0]
    MID = ffn_w1.shape[2]
    S = head_w1.shape[0]
    G = 256 if N >= 10000 else int(batch.max()) + 1
    HD = D // H
    DW = 512 if D == 480 else int(np.ceil(D / 128)) * 128
    assert D % H == 0

    # --- node chunk boundaries: contiguous node ranges, balanced edge counts
    edge_per_node = np.bincount(edst, minlength=N)
    cum = np.concatenate([[0], np.cumsum(edge_per_node)])
    bounds = [0]
    for c in range(1, NCORES):
        target = E * c / NCORES
        bounds.append(int(np.searchsorted(cum, target)))
    bounds.append(N)
    bounds = np.array(bounds, np.int64)

    NPAD = int(np.ceil(max(np.diff(bounds).max(), 128) / 128)) * 128
    NW = NPAD // 128
    NCH = NPAD // 128
    NTAB = NPAD * NCORES

    # global node id -> gather-table row
    node_core = np.searchsorted(bounds, np.arange(N), side="right") - 1
    table_row = NPAD * node_core + (np.arange(N) - bounds[node_core])
    assert table_row.max() < 32768

    order = np.argsort(edst, kind="stable")
    esrc_s = esrc[order]
    edst_s = edst[order]

    # per-core, per-window edge lists
    core_windows = []  # [core][window] -> (src_rows, dst_rel)
    maxT = 1
    for c in range(NCORES):
        lo, hi = bounds[c], bounds[c + 1]
        e0, e1 = np.searchsorted(edst_s, lo), np.searchsorted(edst_s, hi)
        wlists = []
        for w in range(NW):
            nlo = lo + w * 128
            nhi = min(lo + (w + 1) * 128, hi)
            if nlo >= hi:
                wlists.append((np.zeros(0, np.int64), np.zeros(0, np.int64)))
                continue
            a = np.searchsorted(edst_s, nlo)
            b = np.searchsorted(edst_s, nhi)
            wlists.append((table_row[esrc_s[a:b]], edst_s[a:b] - nlo))
            maxT = max(maxT, (b - a + 127) // 128)
        core_windows.append(wlists)
    T = maxT + (maxT % 2)  # even so half-window gathers split cleanly
    EPW = T * 128
    EP = NW * EPW

    # --- per-core edge tensors
    vecs_all = pos[esrc_s] - pos[edst_s]
    d_all = np.linalg.norm(vecs_all, axis=-1)
    sh_all = _sph_l2_np(vecs_all)
    rb_all = _rbf_np(d_all, NB)

    per_core = []
    for c in range(NCORES):
        lo, hi = bounds[c], bounds[c + 1]
        src_rows = np.zeros(EP, np.int64)
        dst_rel = np.full(EP, 300, np.int64)  # 300 -> matches no selector col
        valid = np.zeros(EP, bool)
        orig_pos = np.zeros(EP, np.int64)  # index into sorted edge arrays
        e_base = np.searchsorted(edst_s, lo)
        ofs = e_base
        for w in range(NW):
            sr, dr = core_windows[c][w]
            k = len(sr)
            src_rows[w * EPW:w * EPW + k] = sr
            dst_rel[w * EPW:w * EPW + k] = dr
            valid[w * EPW:w * EPW + k] = True
            orig_pos[w * EPW:w * EPW + k] = np.arange(ofs, ofs + k)
            ofs += k

        shT = np.zeros((16, EP), np.float32)
        rbT = np.zeros((128, EP), np.float32)
        shT[:9, valid] = sh_all[orig_pos[valid]].T
        rbT[:NB, valid] = rb_all[orig_pos[valid]].T

        # selector: [128 edge-in-tile, tiles*128 node cols]
        ntiles = EP // 128
        sel = np.zeros((128, EP), np.float32)
        dr2 = dst_rel.reshape(ntiles, 128)
        for t in range(ntiles):
            m = dr2[t] < 128
            sel[np.nonzero(m)[0], t * 128 + dr2[t][m]] = 1.0

        # node-chunk -> graph selector [128 node-in-chunk, NCH*G cols]
        selg = np.zeros((128, NCH * G), np.float32)
        for ch in range(NCH):
            for j in range(128):
                gid = lo + ch * 128 + j
                if gid < hi:
                    selg[j, ch * G + batch[gid]] = 1.0

        x0 = np.zeros((NPAD, DW), np.float32)
        x0[:hi - lo, :D] = atom_emb[z[lo:hi]]

        per_core.append(dict(
            gidx=_wrap_idx(src_rows.astype(np.int16)),
            shT=shT.astype(ml_dtypes.bfloat16),
            rbT=rbT.astype(ml_dtypes.bfloat16),
            sel=sel.astype(ml_dtypes.bfloat16),
            selg=selg.astype(ml_dtypes.bfloat16),
            x0=x0,
        ))

    # --- weight preparation (|a|-folding, contiguous head-major layout)
    bf = ml_dtypes.bfloat16

    def pad2(a, r, cdim):
        out = np.zeros((r, cdim), np.float32)
        out[:a.shape[0], :a.shape[1]] = a
        return out

    wv_l, wsha_l, w3_l, wo_l, f1_l, f2_l = [], [], [], [], [], []
    sgn_l = []
    for l in range(L):
        a_flat = attn_a[l].reshape(D)  # head-major
        a_abs = np.abs(a_flat)
        a_abs[a_abs < 1e-30] = 1e-30
        sgn = np.where(a_flat >= 0, 1.0, -1.0).astype(np.float32)
        # sign rows, contiguous: head h at cols [h*HD, (h+1)*HD)
        sg = np.zeros((128, DW), np.float32)
        sg[:, :D] = sgn[None, :]
        sgn_l.append(sg)
        wv_l.append(pad2(Wv[l], DW, DW))
        wsha_l.append(pad2(Wsh[l] * a_abs[None, :], 16, DW))
        w3_l.append(pad2(rad_w3[l], FCH, DW))
        wo_l.append(pad2(Wo[l] / a_abs[:, None], DW, DW))
        f1_l.append(pad2(ffn_w1[l], DW, DW))
        f2_l.append(pad2(ffn_w2[l], DW, DW))

    # --- radial MLP pairs: (deg, l0), (l1, l2), (l3, l4), (l5, l5)
    # w1cat: [128, 128] = [w1_A | w1_B]; w2blk block-diag(w2_A, w2_B).
    NP_RAD = (L + 2) // 2  # 4 pairs for L=6
    rad_pairs = []
    mats1 = [deg_w1] + [rad_w1[l] for l in range(L)] + [rad_w1[L - 1]]
    mats2 = [deg_w2] + [rad_w2[l] for l in range(L)] + [rad_w2[L - 1]]
    for p in range(NP_RAD):
        a_i, b_i = 2 * p, 2 * p + 1
        w1cat = np.zeros((128, 2 * FCH), np.float32)
        w1cat[:NB, :FCH] = mats1[a_i]
        w1cat[:NB, FCH:] = mats1[b_i]
        w2blk = np.zeros((2 * FCH, 2 * FCH), np.float32)
        w2blk[:FCH, :FCH] = mats2[a_i]
        w2blk[FCH:, FCH:] = mats2[b_i]
        rad_pairs.append((w1cat, w2blk))

    weights = dict(
        sgn=np.stack(sgn_l).astype(bf),
        wv=np.stack(wv_l).astype(bf), wsha=np.stack(wsha_l).astype(bf),
        w3=np.stack(w3_l).astype(bf), wo=np.stack(wo_l).astype(bf),
        f1=np.stack(f1_l).astype(bf), f2=np.stack(f2_l).astype(bf),
        w1cat=np.stack([a for a, _ in rad_pairs]).astype(bf),
        w2blk=np.stack([b for _, b in rad_pairs]).astype(bf),
        dw3=pad2(deg_w3, FCH, DW).astype(bf),
        wdegsh=pad2(W_deg_sh / AVG_DEG, 16, DW).astype(bf),
        hw1=pad2(head_w1, S, S).astype(bf),
        hw2=pad2(head_w2 / np.sqrt(AVG_NODES), S, S).astype(bf),
    )

    in_maps = []
    for c in range(NCORES):
        m = dict(per_core[c])
        m.update(weights)
        in_maps.append(m)

    meta = dict(
        N=N, E=E, D=D, DW=DW, SH=SH, NB=NB, FCH=FCH, L=L, MID=MID, S=S, G=G,
        HD=HD, NPAD=NPAD, NW=NW, NCH=NCH, T=T, EP=EP, NTAB=NTAB,
        NP_RAD=NP_RAD,
    )
    return meta, in_maps, bounds


# ----------------------------------------------------------------------------
# device program
# ----------------------------------------------------------------------------

def _build_program(meta):
    D, DW, L = meta["D"], meta["DW"], meta["L"]
    SH, NB, FCH = meta["SH"], meta["NB"], meta["FCH"]
    NPAD, NW, NCH, T, EP = meta["NPAD"], meta["NW"], meta["NCH"], meta["T"], meta["EP"]
    NTAB, S, G, HD = meta["NTAB"], meta["S"], meta["G"], meta["HD"]
    NP_RAD = meta["NP_RAD"]
    NK = DW // 128          # 4 contraction chunks of 128
    EPW = T * 128
    GHW = (G + 127) // 128  # graph windows for the head output

    nc = bacc.Bacc("TRN2")

    # ---- parameters
    P = {}
    P["x0"] = nc.declare_dram_parameter("x0", [NPAD, DW], F32, isOutput=False)
    P["rbT"] = nc.declare_dram_parameter("rbT", [128, EP], BF16, isOutput=False)
    P["shT"] = nc.declare_dram_parameter("shT", [16, EP], BF16, isOutput=False)
    P["sel"] = nc.declare_dram_parameter("sel", [128, EP], BF16, isOutput=False)
    P["selg"] = nc.declare_dram_parameter("selg", [128, NCH * G], BF16, isOutput=False)
    P["gidx"] = nc.declare_dram_parameter("gidx", [128, EP // 16], I16, isOutput=False)
    P["sgn"] = nc.declare_dram_parameter("sgn", [L, 128, DW], BF16, isOutput=False)
    P["wv"] = nc.declare_dram_parameter("wv", [L, DW, DW], BF16, isOutput=False)
    P["wsha"] = nc.declare_dram_parameter("wsha", [L, 16, DW], BF16, isOutput=False)
    P["w3"] = nc.declare_dram_parameter("w3", [L, FCH, DW], BF16, isOutput=False)
    P["wo"] = nc.declare_dram_parameter("wo", [L, DW, DW], BF16, isOutput=False)
    P["f1"] = nc.declare_dram_parameter("f1", [L, DW, DW], BF16, isOutput=False)
    P["f2"] = nc.declare_dram_parameter("f2", [L, DW, DW], BF16, isOutput=False)
    P["w1cat"] = nc.declare_dram_parameter("w1cat", [NP_RAD, 128, 2 * FCH], BF16,
                                           isOutput=False)
    P["w2blk"] = nc.declare_dram_parameter("w2blk", [NP_RAD, 2 * FCH, 2 * FCH], BF16,
                                           isOutput=False)
    P["dw3"] = nc.declare_dram_parameter("dw3", [FCH, DW], BF16, isOutput=False)
    P["wdegsh"] = nc.declare_dram_parameter("wdegsh", [16, DW], BF16, isOutput=False)
    P["hw1"] = nc.declare_dram_parameter("hw1", [S, S], BF16, isOutput=False)
    P["hw2"] = nc.declare_dram_parameter("hw2", [S, S], BF16, isOutput=False)
    outp = nc.declare_dram_parameter("outp", [GHW * 128, S], F32, isOutput=True)

    vtab_local = nc.dram_tensor("vtab_local", [NPAD, DW], BF16)
    vtabs = [nc.dram_tensor(f"vtab{i}", [NTAB, DW], BF16, addr_space="Shared")
             for i in range(2)]
    xn_dram = nc.dram_tensor("xn_dram", [NPAD, DW], BF16)
    agg_dram = nc.dram_tensor("agg_dram", [NPAD, DW], BF16)
    # paired radial tables: rows 0:FCH = first member, FCH:2FCH = second
    h2_pair_drams = [nc.dram_tensor(f"h2p_dram{i}", [2 * FCH, EP], BF16)
                     for i in range(NP_RAD)]

    def h2_src(l):
        """(pair_dram, row_offset) supplying layer l (-1 = degree)."""
        idx = l + 1
        return h2_pair_drams[idx // 2], (idx % 2) * FCH

    core_ids = list(range(NCORES))

    with tile.TileContext(nc) as tc, ExitStack() as ctx:
        nc.gpsimd.load_library(library_config.mlp)

        res = ctx.enter_context(tc.tile_pool(name="resident", bufs=1))
        gidx_sb = res.tile([128, EP // 16], I16)
        x_sb = res.tile([128, NCH, DW], F32)
        eps_sb = res.tile([128, 1], F32)
        ones_sb = res.tile([128, 1], BF16)

        nc.sync.dma_start(out=gidx_sb[:], in_=P["gidx"][:])
        for c in range(NCH):
            nc.sync.dma_start(out=x_sb[:, c, :],
                              in_=P["x0"][c * 128:(c + 1) * 128, :])
        nc.vector.memset(eps_sb[:], LN_EPS)
        nc.vector.memset(ones_sb[:], 1.0)

        wpool = ctx.enter_context(tc.tile_pool(name="wpool", bufs=2))

        # ---------- paired radial-MLP table build (dense Silu block) ----------
        def build_h2_pair(p):
            w1_sb = wpool.tile([128, 2 * FCH], BF16, tag="w1", name="w1_sb")
            w2_sb = wpool.tile([2 * FCH, 2 * FCH], BF16, tag="w2", name="w2_sb")
            nc.sync.dma_start(out=w1_sb[:], in_=P["w1cat"][p])
            nc.sync.dma_start(out=w2_sb[:], in_=P["w2blk"][p])
            with tc.tile_pool(name="h2b_ps", bufs=1, space="PSUM") as hbp, \
                 tc.tile_pool(name="h2b_sb", bufs=3) as hbs, \
                 tc.tile_pool(name="h2b_rb", bufs=4) as hbr:
                for c0 in range(0, EP, 512):
                    cw = min(512, EP - c0)
                    rbc = hbr.tile([128, 512], BF16, tag="rbc")
                    nc.sync.dma_start(out=rbc[:, :cw], in_=P["rbT"][:, c0:c0 + cw])
                    h1ps = hbp.tile([128, 512], F32, tag="h1ps")
                    nc.tensor.matmul(h1ps[:, :cw], w1_sb[:], rbc[:, :cw],
                                     start=True, stop=True, skip_group_check=True)
                    h1s = hbs.tile([128, 512], BF16, tag="h1s")
                    nc.scalar.activation(out=h1s[:, :cw], in_=h1ps[:, :cw],
                                         func=AF.Silu)
                    h2ps = hbp.tile([128, 512], F32, tag="h2ps")
                    nc.tensor.matmul(h2ps[:, :cw], w2_sb[:], h1s[:, :cw],
                                     start=True, stop=True, skip_group_check=True)
                    h2s = hbs.tile([128, 512], BF16, tag="h2s")
                    nc.scalar.activation(out=h2s[:, :cw], in_=h2ps[:, :cw],
                                         func=AF.Silu)
                    nc.sync.dma_start(out=h2_pair_drams[p][:, c0:c0 + cw],
                                      in_=h2s[:, :cw])

        # ---------- edge phase (software-pipelined windows) ----------
        def edge_phase(l, build_mids=()):
            """l >= 0: attention layer; l == -1: degree embedding.

            build_mids: pair indices to build, one issued per even window
            boundary (dense Silu blocks overlapping this phase).
            """
            h2_dram, h2_row = h2_src(l)
            wsh_sb = wpool.tile([16, DW], BF16, tag="wsh")
            w3_sb = wpool.tile([FCH, DW], BF16, tag="w3")
            if l >= 0:
                nc.sync.dma_start(out=wsh_sb[:], in_=P["wsha"][l])
                nc.sync.dma_start(out=w3_sb[:], in_=P["w3"][l])
                sgn_sb = wpool.tile([128, DW], BF16, tag="sgn")
                nc.sync.dma_start(out=sgn_sb[:], in_=P["sgn"][l])
                vtab = vtabs[l % 2]
            else:
                nc.sync.dma_start(out=wsh_sb[:], in_=P["wdegsh"][:])
                nc.sync.dma_start(out=w3_sb[:], in_=P["dw3"][:])
                vtab = None

            with tc.tile_pool(name="eps_shw", bufs=2, space="PSUM") as psA, \
                 tc.tile_pool(name="eps_rad", bufs=2, space="PSUM") as psB, \
                 tc.tile_pool(name="wps", bufs=2, space="PSUM") as wps, \
                 tc.tile_pool(name="esb", bufs=3) as esb, \
                 tc.tile_pool(name="amp", bufs=2) as amp, \
                 tc.tile_pool(name="vg", bufs=3) as vgp, \
                 tc.tile_pool(name="selp", bufs=2) as selp, \
                 tc.tile_pool(name="aggp", bufs=2) as aggp:

                win = {}

                def load_window(w):
                    d = {}
                    d["sel"] = selp.tile([128, EPW], BF16, tag="selw", name="sel_w")
                    nc.sync.dma_start(out=d["sel"][:],
                                      in_=P["sel"][:, w * EPW:(w + 1) * EPW])
                    d["shT"] = selp.tile([16, EPW], BF16, tag="shtw", name="shT_w")
                    nc.sync.dma_start(out=d["shT"][:],
                                      in_=P["shT"][:, w * EPW:(w + 1) * EPW])
                    d["h2T"] = selp.tile([FCH, EPW], BF16, tag="h2tw", name="h2T_w")
                    nc.sync.dma_start(
                        out=d["h2T"][:],
                        in_=h2_dram[h2_row:h2_row + FCH, w * EPW:(w + 1) * EPW])
                    if l >= 0:
                        half = EPW // 2
                        d["vbuf"] = vgp.tile([128, T * DW], BF16, tag="vbuf", name="vbuf_w")
                        for gi in range(2):
                            i0 = w * EPW + gi * half
                            nc.gpsimd.dma_gather(
                                out_ap=d["vbuf"][:, gi * (half // 128) * DW:
                                                 (gi + 1) * (half // 128) * DW
                                                 ].rearrange("p (j e) -> p j e", e=DW),
                                in_ap=vtab[:],
                                idxs_ap=gidx_sb[:, i0 // 16:(i0 + half) // 16],
                                num_idxs=half, num_idxs_reg=half,
                                elem_size=DW, single_packet=False)
                    return d

                def stageA(w):
                    if w == 0:
                        win[0] = load_window(0)
                        if NW > 1:
                            win[1] = load_window(1)
                    if w + 2 < NW:
                        win[w + 2] = load_window(w + 2)
                    d = win[w]
                    d["am"] = amp.tile([128, T * DW], BF16, tag="am", name="am_w")
                    if l >= 0:
                        d["logit"] = amp.tile([128, T * H], F32, tag="logit", name="logit_w")
                    else:
                        d["psw"] = wps.tile([128, 512], F32, tag="psw", name="psw_w")
                    for t in range(T):
                        shw_ps = psA.tile([128, DW], F32, tag="shw_ps")
                        nc.tensor.matmul(shw_ps[:],
                                         d["shT"][:SH, t * 128:(t + 1) * 128],
                                         wsh_sb[:SH, :], start=True, stop=True,
                                         skip_group_check=True)
                        shw_sb = esb.tile([128, DW], BF16, tag="shw_sb")
                        nc.scalar.activation(out=shw_sb[:, :D], in_=shw_ps[:, :D],
                                             func=AF.Copy)
                        rad_ps = psB.tile([128, DW], F32, tag="rad_ps")
                        nc.tensor.matmul(rad_ps[:],
                                         d["h2T"][:, t * 128:(t + 1) * 128],
                                         w3_sb[:], start=True, stop=True,
                                         skip_group_check=True)
                        am_t = d["am"][:, t * DW:t * DW + D]
                        if l >= 0:
                            rad_sb = esb.tile([128, DW], BF16, tag="rad_sb")
                            nc.scalar.activation(out=rad_sb[:, :D],
                                                 in_=rad_ps[:, :D], func=AF.Copy)
                            tt = esb.tile([128, DW], BF16, tag="tt")
                            nc.vector.tensor_tensor(
                                out=tt[:, :D],
                                in0=d["vbuf"][:, t * DW:t * DW + D],
                                in1=shw_sb[:, :D], op=OP.mult)
                            nc.vector.tensor_tensor(
                                out=am_t, in0=tt[:, :D], in1=rad_sb[:, :D],
                                op=OP.mult)
                            junk = esb.tile([128, DW], BF16, tag="junk")
                            nc.scalar.activation(
                                out=junk[:, :D], in_=am_t, func=AF.Lrelu,
                                alpha=0.2)
                            scr = esb.tile([128, DW], BF16, tag="scr")
                            for h in range(H):
                                nc.vector.scalar_tensor_tensor(
                                    out=scr[:, h * HD:(h + 1) * HD],
                                    in0=junk[:, h * HD:(h + 1) * HD], scalar=1.0,
                                    in1=sgn_sb[:, h * HD:(h + 1) * HD],
                                    op0=OP.mult, op1=OP.mult,
                                    accum_out=d["logit"][:, t * H + h:t * H + h + 1])
                        else:
                            nc.vector.tensor_tensor(
                                out=am_t, in0=shw_sb[:, :D],
                                in1=rad_ps[:, :D], op=OP.mult)
                            nc.tensor.matmul(d["psw"][:, :D],
                                             d["sel"][:, t * 128:(t + 1) * 128],
                                             am_t,
                                             start=(t == 0), stop=(t == T - 1),
                                             skip_group_check=True)

                def stageB(w):
                    d = win.pop(w)
                    if l < 0:
                        # x = emb + deg
                        nc.vector.scalar_tensor_tensor(
                            out=x_sb[:, w, :D], in0=d["psw"][:, :D], scalar=1.0,
                            in1=x_sb[:, w, :D], op0=OP.mult, op1=OP.add)
                        return
                    psw = wps.tile([128, 512], F32, tag="psw")
                    pss = psw[:, D:D + H]
                    ex = amp.tile([128, T * H], BF16, tag="ex")
                    nc.scalar.activation(out=ex[:], in_=d["logit"][:], func=AF.Exp)
                    for t in range(T):
                        am3 = d["am"][:, t * DW:t * DW + D].rearrange(
                            "p (h s) -> p h s", s=HD)
                        ex3 = ex[:, t * H:(t + 1) * H].rearrange(
                            "p (h one) -> p h one", one=1)
                        amw = esb.tile([128, DW], BF16, tag="amw")
                        amw3 = amw[:, :D].rearrange("p (h s) -> p h s", s=HD)
                        nc.vector.tensor_tensor(
                            out=amw3, in0=am3,
                            in1=ex3.to_broadcast([128, H, HD]), op=OP.mult)
                        nc.tensor.matmul(psw[:, :D],
                                         d["sel"][:, t * 128:(t + 1) * 128],
                                         amw[:, :D],
                                         start=(t == 0), stop=(t == T - 1),
                                         skip_group_check=True)
                        nc.tensor.matmul(pss,
                                         d["sel"][:, t * 128:(t + 1) * 128],
                                         ex[:, t * H:(t + 1) * H],
                                         start=(t == 0), stop=(t == T - 1),
                                         skip_group_check=True)
                    # window epilogue
                    rs = esb.tile([128, H], F32, tag="rs")
                    nc.vector.tensor_scalar(
                        out=rs[:], in0=pss, scalar1=SEG_EPS,
                        scalar2=None, op0=OP.add)
                    nc.vector.reciprocal(out=rs[:], in_=rs[:])
                    aggs = aggp.tile([128, DW], BF16, tag="aggs")
                    for h in range(H):
                        nc.vector.tensor_scalar(
                            out=aggs[:, h * HD:(h + 1) * HD],
                            in0=psw[:, h * HD:(h + 1) * HD],
                            scalar1=rs[:, h:h + 1], scalar2=None, op0=OP.mult)
                    if D < DW:
                        nc.vector.memset(aggs[:, D:], 0.0)
                    nc.sync.dma_start(out=agg_dram[w * 128:(w + 1) * 128, :],
                                      in_=aggs[:])

                for w in range(NW + 1):
                    if w < NW:
                        stageA(w)
                    if w >= 1:
                        stageB(w - 1)
                    if w == 2 and len(build_mids) > 0:
                        build_h2_pair(build_mids[0])

        # ---------- LN + transpose helper ----------
        def ln_to_dram(ncols):
            """LN(x[:, :ncols]) -> xn_dram (bf16, padded cols dirty-but-masked)."""
            with tc.tile_pool(name="lnp", bufs=2) as lnp:
                for ch in range(NCH):
                    st6 = lnp.tile([128, 6], F32, tag="st6")
                    nc.vector.bn_stats(out=st6[:], in_=x_sb[:, ch, :ncols])
                    mv = lnp.tile([128, 2], F32, tag="mv")
                    nc.vector.bn_aggr(out=mv[:], in_=st6[:])
                    r = lnp.tile([128, 1], F32, tag="r")
                    nc.scalar.activation(out=r[:], in_=mv[:, 1:2], func=AF.Sqrt,
                                         bias=eps_sb[:], scale=1.0)
                    nc.vector.reciprocal(out=r[:], in_=r[:])
                    xn = lnp.tile([128, DW], BF16, tag="xn")
                    nc.vector.tensor_scalar(
                        out=xn[:, :ncols], in0=x_sb[:, ch, :ncols],
                        scalar1=mv[:, 0:1], scalar2=r[:],
                        op0=OP.subtract, op1=OP.mult)
                    if ncols < DW:
                        nc.vector.memset(xn[:, ncols:], 0.0)
                    nc.sync.dma_start(out=xn_dram[ch * 128:(ch + 1) * 128, :], in_=xn[:])

        def transpose_from_dram(src_dram, dst_sb):
            for k in range(NK):
                nc.sync.dma_start_transpose(
                    out=dst_sb[:, k, :NPAD],
                    in_=src_dram[:, k * 128:(k + 1) * 128])

        # ---------- matmul x[chunk] @ W  (+ optional x update) ----------
        def rowmm_update(xt_sb, w_dram_3d, update):
            """x_sb[:,ch,:] (+)= (xt)^T @ W; update=True adds into x."""
            wk = wpool.tile([128, NK, DW], BF16, tag="wk")
            for k in range(NK):
                nc.sync.dma_start(out=wk[:, k, :],
                                  in_=w_dram_3d[k * 128:(k + 1) * 128, :])
            with tc.tile_pool(name="rmm", bufs=2, space="PSUM") as pps, \
                 tc.tile_pool(name="rmm_sb", bufs=2) as osb:
                for ch in range(NCH):
                    ps = pps.tile([128, DW], F32, tag="ps")
                    for k in range(NK):
                        nc.tensor.matmul(ps[:],
                                         xt_sb[:, k, ch * 128:(ch + 1) * 128],
                                         wk[:, k, :],
                                         start=(k == 0), stop=(k == NK - 1))
                    if update:
                        nc.vector.scalar_tensor_tensor(
                            out=x_sb[:, ch, :], in0=ps[:], scalar=1.0,
                            in1=x_sb[:, ch, :], op0=OP.mult, op1=OP.add)
                    else:
                        vrow = osb.tile([128, DW], BF16, tag="vrow")
                        nc.scalar.activation(out=vrow[:], in_=ps[:], func=AF.Copy)
                        nc.sync.dma_start(
                            out=vtab_local[ch * 128:(ch + 1) * 128, :], in_=vrow[:])

        xt_sb = res.tile([128, NK, NPAD], BF16)
        mid_sb = res.tile([128, NK, NPAD], BF16)

        # ================= program =================
        build_h2_pair(0)
        edge_phase(-1, build_mids=(1,))

        for l in range(L):
            # LN1 -> v table -> allgather
            ln_to_dram(D)
            transpose_from_dram(xn_dram, xt_sb)
            rowmm_update(xt_sb, P["wv"][l], update=False)
            nc.gpsimd.collective_compute(
                "AllGather", OP.bypass,
                ins=[vtab_local[:]], outs=[vtabs[l % 2][:]],
                replica_groups=[core_ids])
            bm = l + 2
            edge_phase(l, build_mids=(bm,) if bm < NP_RAD else ())
            # x += agg @ Wo
            transpose_from_dram(agg_dram, xt_sb)
            rowmm_update(xt_sb, P["wo"][l], update=True)
            # ffn
            ln_to_dram(D)
            transpose_from_dram(xn_dram, xt_sb)
            f1k = wpool.tile([128, NK, DW], BF16, tag="wk")
            for k in range(NK):
                nc.sync.dma_start(out=f1k[:, k, :],
                                  in_=P["f1"][l][k * 128:(k + 1) * 128, :])
            with tc.tile_pool(name="ffn_ps", bufs=2, space="PSUM") as fps:
                for mch in range(NK):
                    for n0 in range(0, NPAD, 512):
                        nw_ = min(512, NPAD - n0)
                        ps = fps.tile([128, 512], F32, tag="fps")
                        for k in range(NK):
                            nc.tensor.matmul(
                                ps[:, :nw_],
                                f1k[:, k, mch * 128:(mch + 1) * 128],
                                xt_sb[:, k, n0:n0 + nw_],
                                start=(k == 0), stop=(k == NK - 1))
                        nc.scalar.activation(out=mid_sb[:, mch, n0:n0 + nw_],
                                             in_=ps[:, :nw_], func=AF.Silu)
            rowmm_update(mid_sb, P["f2"][l], update=True)

        # ================= output head =================
        with tc.tile_pool(name="head", bufs=2) as hp, \
             tc.tile_pool(name="head_ps", bufs=2, space="PSUM") as hps, \
             tc.tile_pool(name="head_ps1", bufs=1, space="PSUM") as hps1, \
             tc.tile_pool(name="head_res", bufs=1) as hr:
            ident = hr.tile([128, 128], BF16)
            from concourse.masks import make_identity
            make_identity(nc, ident[:])
            sT = hr.tile([128, NPAD], BF16)
            hw1_sb = hr.tile([S, S], BF16)
            hw2_sb = hr.tile([S, S], BF16)
            selg_sb = hr.tile([128, NCH * G], BF16)
            nc.sync.dma_start(out=hw1_sb[:], in_=P["hw1"][:])
            nc.sync.dma_start(out=hw2_sb[:], in_=P["hw2"][:])
            nc.sync.dma_start(out=selg_sb[:], in_=P["selg"][:])
            for ch in range(NCH):
                st6 = hp.tile([128, 6], F32, tag="hst6")
                nc.vector.bn_stats(out=st6[:], in_=x_sb[:, ch, :S])
                mv = hp.tile([128, 2], F32, tag="hmv")
                nc.vector.bn_aggr(out=mv[:], in_=st6[:])
                r = hp.tile([128, 1], F32, tag="hr")
                nc.scalar.activation(out=r[:], in_=mv[:, 1:2], func=AF.Sqrt,
                                     bias=eps_sb[:], scale=1.0)
                nc.vector.reciprocal(out=r[:], in_=r[:])
                s_sb = hp.tile([128, S], BF16, tag="s_sb")
                nc.vector.tensor_scalar(
                    out=s_sb[:], in0=x_sb[:, ch, :S],
                    scalar1=mv[:, 0:1], scalar2=r[:],
                    op0=OP.subtract, op1=OP.mult)
                tps = hps.tile([128, 128], BF16, tag="tps")
                nc.tensor.transpose(tps[:], s_sb[:], ident[:])
                nc.scalar.activation(out=sT[:, ch * 128:(ch + 1) * 128], in_=tps[:],
                                     func=AF.Copy)
            # mid = silu(s @ hw1): midT = hw1^T-stationary
            mh_sT = hr.tile([128, NPAD], BF16)
            for n0 in range(0, NPAD, 512):
                nw_ = min(512, NPAD - n0)
                ps = hps.tile([128, 512], F32, tag="hmps")
                nc.tensor.matmul(ps[:, :nw_], hw1_sb[:], sT[:, n0:n0 + nw_],
                                 start=True, stop=True)
                nc.scalar.activation(out=mh_sT[:, n0:n0 + nw_], in_=ps[:, :nw_],
                                     func=AF.Silu)
            outg_ps = [hps1.tile([128, S], F32, tag=f"outg{gw}", name=f"outg{gw}")
                       for gw in range(GHW)]
            for ch in range(NCH):
                hrow_ps = hps.tile([128, S], F32, tag="hrow")
                nc.tensor.matmul(hrow_ps[:], mh_sT[:, ch * 128:(ch + 1) * 128],
                                 hw2_sb[:], start=True, stop=True)
                h_sb = hp.tile([128, S], BF16, tag="h_sb")
                nc.scalar.activation(out=h_sb[:], in_=hrow_ps[:], func=AF.Copy)
                for gw in range(GHW):
                    gn = min(128, G - gw * 128)
                    nc.tensor.matmul(outg_ps[gw][:gn, :],
                                     selg_sb[:, ch * G + gw * 128: ch * G + gw * 128 + gn],
                                     h_sb[:],
                                     start=(ch == 0), stop=(ch == NCH - 1),
                                     skip_group_check=True)
            for gw in range(GHW):
                og = hp.tile([128, S], F32, tag="og")
                nc.vector.tensor_copy(out=og[:], in_=outg_ps[gw][:])
                nc.sync.dma_start(out=outp[gw * 128:(gw + 1) * 128, :], in_=og[:])

    nc.compile()
    return nc


def _get_program(meta):
    key = tuple(sorted(meta.items()))
    if key not in _program_cache:
        _program_cache[key] = _build_program(meta)
    return _program_cache[key]


# ----------------------------------------------------------------------------
# entry point
# ----------------------------------------------------------------------------

def kernel(**inputs):
    meta, in_maps, bounds = _prepare(inputs)
    nc = _get_program(meta)
    from concourse import bass2jax
    results = bass2jax.run_bass_via_pjrt(nc, in_maps, n_cores=NCORES)
    G, S = meta["G"], meta["S"]
    out = np.zeros((G, S), np.float32)
    for c in range(NCORES):
        out += np.asarray(results[c]["outp"])[:G, :S]
    return out

